# revision 1
# baseline (speedup 1.0000x reference)
"""Localized (block-diagonal windowed) self-attention + residual + LayerNorm
on 8 Trainium2 NeuronCores.

Problem (hardcoded): x [B=4, S=4096, D=1024], H=16 heads, K=64 head dim,
num_window=8 -> window length Sw=512. Per (batch, window) block:
    q/k/v = xw @ W* + b*          [512, 16, 64]
    scores = q k^T / 8 per head   [512, 512]
    attn = softmax(scores)
    ctx = attn @ v
    attn_out = ctx @ Wo + bo
    out = LayerNorm(x + attn_out) * gamma + beta   (eps=1e-3)

Sharding: pure data parallelism over the 32 (batch, window) blocks, 4 per
core; weights replicated. No collectives.

Device layout choices (all matmuls bf16, fp32 PSUM accumulation):
  - xT (D-major) prepared on host -> qT, kT (hk-major) and v (s-major)
    come straight out of matmuls with no device transposes.
  - Attention computed k-major: scoresT[ks, q] = kT_h^T-slice @ qT_h, so
    softmax exp rides the ACT engine psum->sbuf and the ctx matmul
    lhsT = [v_h | ones] produces ctxT (hk-major, exactly what the output
    projection needs as lhsT) with the softmax denominator replicated in
    psum rows 64:128 -> reciprocal + multiply, no partition broadcasts.
  - exp scale 1/sqrt(K) folded into kT; rsqrt for LN via Ln+Exp (same ACT
    table set as attention's Exp -> no table switches).
  - bo folded into x on host; gamma/beta applied on host after the kernel
    (exact: same op order as the reference).
"""

import numpy as np
import ml_dtypes

import concourse.bacc as bacc
import concourse.mybir as mybir
from concourse.tile import TileContext
from concourse import bass_utils

F32 = mybir.dt.float32
BF16 = mybir.dt.bfloat16
ALU = mybir.AluOpType
ACTF = mybir.ActivationFunctionType
AX = mybir.AxisListType

B, S, D, H, K = 4, 4096, 1024, 16, 64
NW = 8            # windows per sequence
SW = S // NW      # 512
NCORES = 8
NBLK = B * NW     # 32 (batch, window) blocks
WPC = NBLK // NCORES  # 4 blocks per core
DC = D // 128     # 8 contraction chunks
HC = (H * K) // 128   # 8 hk chunks
SC = SW // 128    # 4 s chunks per window

TRACE = False          # test.py sets True to capture an NTFF profile
LAST_RESULT = None     # BassKernelResults of the last run (for timing)

_cached_nc = None


def _build_nc(reps=1):
    # reps > 1 repeats the whole per-window computation (same inputs/outputs)
    # to amplify device time for wall-clock measurement; reps=1 for real runs.
    nc = bacc.Bacc(None, target_bir_lowering=False, debug=False)

    xT_in = nc.dram_tensor("xt", [WPC, DC, 128, SW], BF16, kind="ExternalInput")
    x_in = nc.dram_tensor("x", [WPC, SC, 128, D], F32, kind="ExternalInput")
    wq_in = nc.dram_tensor("wq", [DC, 128, D], BF16, kind="ExternalInput")
    wk_in = nc.dram_tensor("wk", [DC, 128, D], BF16, kind="ExternalInput")
    wv_in = nc.dram_tensor("wv", [DC, 128, D], BF16, kind="ExternalInput")
    wo_in = nc.dram_tensor("wo", [HC, 128, D], BF16, kind="ExternalInput")
    bq_in = nc.dram_tensor("bq", [128, HC], F32, kind="ExternalInput")
    bk_in = nc.dram_tensor("bk", [128, HC], F32, kind="ExternalInput")  # pre-scaled by 1/8
    bv_in = nc.dram_tensor("bv", [128, D], F32, kind="ExternalInput")   # pre-broadcast
    out = nc.dram_tensor("out", [WPC, SC, 128, D], F32, kind="ExternalOutput")

    with TileContext(nc) as tc:
        with tc.tile_pool(name="const", bufs=1) as cpool, \
             tc.tile_pool(name="wts", bufs=1) as wpool, \
             tc.tile_pool(name="xt", bufs=2) as xt_pool, \
             tc.tile_pool(name="xnat", bufs=3) as xn_pool, \
             tc.tile_pool(name="qk", bufs=2) as qk_pool, \
             tc.tile_pool(name="vv", bufs=5) as v_pool, \
             tc.tile_pool(name="et", bufs=4) as e_pool, \
             tc.tile_pool(name="rcp", bufs=2) as r_pool, \
             tc.tile_pool(name="ctx", bufs=2) as c_pool, \
             tc.tile_pool(name="yy", bufs=5) as y_pool, \
             tc.tile_pool(name="oo", bufs=2) as o_pool, \
             tc.tile_pool(name="st", bufs=4) as s_pool, \
             tc.tile_pool(name="ps_proj", bufs=2, space="PSUM") as ps_proj, \
             tc.tile_pool(name="ps_sc", bufs=2, space="PSUM") as ps_sc, \
             tc.tile_pool(name="ps_acc", bufs=2, space="PSUM") as ps_acc:

            # ---- persistent constants ----
            wq_sb = wpool.tile([128, DC, D], BF16, tag="wq")
            nc.sync.dma_start(wq_sb, wq_in.rearrange("c p d -> p c d"))
            wk_sb = wpool.tile([128, DC, D], BF16, tag="wk")
            nc.sync.dma_start(wk_sb, wk_in.rearrange("c p d -> p c d"))
            wv_sb = wpool.tile([128, DC, D], BF16, tag="wv")
            nc.sync.dma_start(wv_sb, wv_in.rearrange("c p d -> p c d"))
            wo_sb = wpool.tile([128, HC, D], BF16, tag="wo")
            nc.sync.dma_start(wo_sb, wo_in.rearrange("c p d -> p c d"))
            bq_sb = cpool.tile([128, HC], F32, tag="bq")
            nc.sync.dma_start(bq_sb, bq_in[:, :])
            bk_sb = cpool.tile([128, HC], F32, tag="bk")
            nc.sync.dma_start(bk_sb, bk_in[:, :])
            bv_sb = cpool.tile([128, D], F32, tag="bv")
            nc.sync.dma_start(bv_sb, bv_in[:, :])
            eps_sb = cpool.tile([128, 1], F32, tag="eps")
            nc.vector.memset(eps_sb, 1e-3)

            for w in [wi for _ in range(reps) for wi in range(WPC)]:
                # ---- load xT for this window ----
                xT_t = xt_pool.tile([128, DC, SW], BF16, tag="xT")
                nc.sync.dma_start(xT_t, xT_in[w].rearrange("c p s -> p c s"))

                # ---- qT, kT projections: [hk, s] hk-major ----
                qT_t = qk_pool.tile([128, HC, SW], BF16, tag="qT")
                kT_t = qk_pool.tile([128, HC, SW], BF16, tag="kT")
                for j in range(HC):
                    pq = ps_proj.tile([128, 512], F32, tag="pp")
                    for i in range(DC):
                        nc.tensor.matmul(pq, lhsT=wq_sb[:, i, j * 128:(j + 1) * 128],
                                         rhs=xT_t[:, i, :],
                                         start=(i == 0), stop=(i == DC - 1))
                    nc.any.tensor_scalar(qT_t[:, j, :], pq,
                                         bq_sb[:, j:j + 1], None, ALU.add)
                    pk = ps_proj.tile([128, 512], F32, tag="pp")
                    for i in range(DC):
                        nc.tensor.matmul(pk, lhsT=wk_sb[:, i, j * 128:(j + 1) * 128],
                                         rhs=xT_t[:, i, :],
                                         start=(i == 0), stop=(i == DC - 1))
                    # kT = k_psum * 0.125 + bk*0.125  (bk pre-scaled on host)
                    nc.any.tensor_scalar(kT_t[:, j, :], pk, 0.125,
                                         bk_sb[:, j:j + 1], ALU.mult, ALU.add)

                # ---- v projection: [s, hk] natural, per-head [v_h | ones] ----
                v_tiles = []
                for m in range(SC):
                    vt = v_pool.tile([128, H, 128], BF16, tag="v")
                    for half in range(2):
                        pv = ps_proj.tile([128, 512], F32, tag="pp")
                        for i in range(DC):
                            nc.tensor.matmul(
                                pv, lhsT=xT_t[:, i, m * 128:(m + 1) * 128],
                                rhs=wv_sb[:, i, half * 512:(half + 1) * 512],
                                start=(i == 0), stop=(i == DC - 1))
                        nc.vector.tensor_tensor(
                            vt[:, half * 8:(half + 1) * 8, 0:64],
                            pv.rearrange("p (c k) -> p c k", k=64),
                            bv_sb.rearrange("p (c k) -> p c k", k=64)[
                                :, half * 8:(half + 1) * 8, :], op=ALU.add)
                    nc.vector.memset(vt[:, :, 64:128], 1.0)
                    v_tiles.append(vt)

                # ---- attention per head (k-major, fused denominator) ----
                ctx_t = c_pool.tile([128, HC, SW], BF16, tag="ctx")
                for h in range(H):
                    j, po = h // 2, (h % 2) * 64
                    cps = ps_acc.tile([128, 512], F32, tag="acc")
                    for k2 in range(SC // 2):
                        sps = ps_sc.tile([128, 2, 512], F32, tag="sps")
                        for u in range(2):
                            ks = 2 * k2 + u
                            nc.tensor.matmul(
                                sps[:, u, :],
                                lhsT=kT_t[po:po + 64, j, ks * 128:(ks + 1) * 128],
                                rhs=qT_t[po:po + 64, j, :], start=True, stop=True)
                        et = e_pool.tile([128, 2, 512], BF16, tag="exp")
                        nc.scalar.activation(et, sps, ACTF.Exp)
                        for u in range(2):
                            ks = 2 * k2 + u
                            # lhsT = [v_h (64) | ones (64)], contiguous
                            nc.tensor.matmul(cps, lhsT=v_tiles[ks][:, h, :],
                                             rhs=et[:, u, :],
                                             start=(ks == 0), stop=(ks == SC - 1))
                    rb = r_pool.tile([64, 512], F32, tag="rcp")
                    nc.vector.reciprocal(rb, cps[64:128, :])
                    nc.vector.tensor_tensor(ctx_t[po:po + 64, j, :], cps[0:64, :],
                                            rb, op=ALU.mult)

                # ---- output projection + residual + layernorm per s-chunk ----
                # Stats batched per window so the ACT Sqrt runs once per
                # window (minimizes activation-table switches vs Exp).
                y_ts = []
                negmu4 = s_pool.tile([128, SC], F32, tag="negmu")
                var4 = s_pool.tile([128, SC], F32, tag="var")
                for m in range(SC):
                    x_t = xn_pool.tile([128, D], F32, tag="xn")
                    nc.sync.dma_start(x_t, x_in[w, m])
                    y_t = y_pool.tile([128, D], F32, tag="y")
                    y_ts.append(y_t)
                    ysum = s_pool.tile([128, 2], F32, tag="ysum")
                    for half in range(2):
                        pout = ps_acc.tile([128, 512], F32, tag="acc")
                        for j in range(HC):
                            nc.tensor.matmul(
                                pout, lhsT=ctx_t[:, j, m * 128:(m + 1) * 128],
                                rhs=wo_sb[:, j, half * 512:(half + 1) * 512],
                                start=(j == 0), stop=(j == HC - 1))
                        # y = x + attn_out, with fused row-sum accumulation
                        nc.vector.scalar_tensor_tensor(
                            y_t[:, half * 512:(half + 1) * 512],
                            x_t[:, half * 512:(half + 1) * 512], 1.0, pout,
                            ALU.mult, ALU.add,
                            accum_out=ysum[:, half:half + 1])
                    nc.vector.tensor_scalar(negmu4[:, m:m + 1], ysum[:, 0:1],
                                            ysum[:, 1:2], -1.0 / D,
                                            ALU.add, ALU.mult)
                    # sum(y^2) on DVE (scratch write into the dead x tile)
                    sumsq = s_pool.tile([128, 1], F32, tag="sumsq")
                    nc.vector.scalar_tensor_tensor(x_t, y_t, 1.0, y_t,
                                                   ALU.mult, ALU.mult,
                                                   accum_out=sumsq)
                    musq = s_pool.tile([128, 1], F32, tag="musq")
                    nc.vector.tensor_tensor(musq, negmu4[:, m:m + 1],
                                            negmu4[:, m:m + 1], op=ALU.mult)
                    nc.vector.tensor_scalar(var4[:, m:m + 1], sumsq, 1.0 / D,
                                            musq, ALU.mult, ALU.subtract)
                # rstd = 1 / sqrt(var + eps): one ACT Sqrt per window
                sd4 = s_pool.tile([128, SC], F32, tag="sd4")
                nc.scalar.activation(sd4, var4, ACTF.Sqrt, bias=eps_sb[:, 0:1])
                rstd4 = s_pool.tile([128, SC], F32, tag="rstd4")
                nc.vector.reciprocal(rstd4, sd4)
                for m in range(SC):
                    o_t = o_pool.tile([128, D], F32, tag="o")
                    nc.vector.tensor_scalar(o_t, y_ts[m], negmu4[:, m:m + 1],
                                            rstd4[:, m:m + 1],
                                            ALU.add, ALU.mult)
                    nc.sync.dma_start(out[w, m], o_t)

    nc.compile()
    return nc


def _get_nc():
    global _cached_nc
    if _cached_nc is None:
        _cached_nc = _build_nc()
    return _cached_nc


def kernel(x, Wq, bq, Wk, bk, Wv, bv, Wo, bo, gamma, beta, num_window):
    global LAST_RESULT
    x = np.ascontiguousarray(np.asarray(x, dtype=np.float32))
    Wq = np.asarray(Wq, np.float32)
    Wk = np.asarray(Wk, np.float32)
    Wv = np.asarray(Wv, np.float32)
    Wo = np.asarray(Wo, np.float32)
    bq = np.asarray(bq, np.float32).reshape(H * K)
    bk = np.asarray(bk, np.float32).reshape(H * K)
    bv = np.asarray(bv, np.float32).reshape(H * K)
    bo = np.asarray(bo, np.float32).reshape(D)
    gamma = np.asarray(gamma, np.float32).reshape(D)
    beta = np.asarray(beta, np.float32).reshape(D)
    assert int(num_window) == NW, f"kernel compiled for num_window={NW}"
    assert x.shape == (B, S, D)

    bf16 = ml_dtypes.bfloat16
    # Blocks: (b, w) -> flat index b*NW + w; core c owns blocks [c*WPC, (c+1)*WPC)
    xb = x.reshape(NBLK, SW, D)
    if np.any(bo):
        xb = xb + bo  # fold output-projection bias into the residual input
    x_nat = np.ascontiguousarray(xb.reshape(NBLK, SC, 128, D), np.float32)
    xT = np.ascontiguousarray(
        xb.transpose(0, 2, 1).reshape(NBLK, DC, 128, SW)).astype(bf16)

    shared = {
        "wq": np.ascontiguousarray(Wq.reshape(D, H * K).reshape(DC, 128, D)).astype(bf16),
        "wk": np.ascontiguousarray(Wk.reshape(D, H * K).reshape(DC, 128, D)).astype(bf16),
        "wv": np.ascontiguousarray(Wv.reshape(D, H * K).reshape(DC, 128, D)).astype(bf16),
        "wo": np.ascontiguousarray(Wo.reshape(H * K, D).reshape(HC, 128, D)).astype(bf16),
        "bq": np.ascontiguousarray(bq.reshape(HC, 128).T, np.float32),
        "bk": np.ascontiguousarray((bk * 0.125).reshape(HC, 128).T, np.float32),
        "bv": np.ascontiguousarray(np.broadcast_to(bv, (128, D)), np.float32),
    }
    in_maps = []
    for c in range(NCORES):
        m = dict(shared)
        m["xt"] = np.ascontiguousarray(xT[c * WPC:(c + 1) * WPC])
        m["x"] = np.ascontiguousarray(x_nat[c * WPC:(c + 1) * WPC])
        in_maps.append(m)

    nc = _get_nc()
    res = bass_utils.run_bass_kernel_spmd(
        nc, in_maps, core_ids=list(range(NCORES)), trace=TRACE)
    LAST_RESULT = res

    y = np.empty((NBLK, SC, 128, D), np.float32)
    for c in range(NCORES):
        y[c * WPC:(c + 1) * WPC] = res.results[c]["out"]
    y = y.reshape(B, S, D)
    if np.any(gamma != 1.0) or np.any(beta):
        y = y * gamma + beta
    return y

